# revision 75
# baseline (speedup 1.0000x reference)
"""DensePatchAttention Trainium2 kernel.

Full (unsharded) inputs -> full output. Internally shards across 8
NeuronCores as (batch b in 0..3) x (head-group g in 0..1, 4 heads each).

Reference computation (per batch):
  q = 1x1conv(x, Wq) + bq                  [256, 128, 128]
  k = 8x8/s8conv(x, Wk) + bk               [256, 16, 16]
  v = 8x8/s8conv(x, Wv) + bv               [256, 16, 16]
  per head h (c=32 channels, channel = c_idx*8 + h):
    dots = q_h^T k_h        [HW=16384, K=256]
    attn = softmax(dots)
    out_h = attn @ v_h      [16384, 32]

Device program (head-local channel row m = h_local*32 + c, all big
tensors bf16; weights pre-arranged on host so every DMA is contiguous):

The Act engine is the hard floor (~133us of exp), so the schedule is
built around keeping it busy from the earliest possible moment:

  - x arrives as 4 quarters [chunk, pos-half]; k conv runs per
    (chunk, key-half) so kb0 dots/exp start ~31us in, before x
    has even fully landed.  Dep-pinned dummy matmuls keep the PE DVFS
    clock up through the DMA gaps.
  - first SPLIT tiles: A+exp per key-half (split-exp), their e tiles
    stashed in a one-shot pool; q for the next tile is prefetched
    inside each tile.  The kb1 k-conv chunks are woven into phase A
    and the v-conv chunks into phase B so the PE computes them in the
    Act-paced slack.  C for those tiles catches up during steady
    state (one self-contained pair per tile).
  - steady tiles: 4x(A[128,1024] via zero-padded k_ext, contraction
    64) -> exp -> C (lhsT = vt blocks [vT(32)|ones(1)|zeros(31)], two
    heads per PSUM tile at partition bases 0/64, accumulated over kb).
    The ones column makes the softmax denominator ride the C matmul
    for free; the zero columns define all po2 partitions so the
    staging copy is clean.
  - per pair: one DVE copy po2[128,512] f32 -> bf16, DMA to DRAM.
  - softmax division (num / rowsum) happens ON HOST after gather --
    no on-device reciprocal/divide at all.
"""

import numpy as np

try:
    import concourse.bass as bass  # noqa: F401
except ImportError:  # pragma: no cover
    import sys
    sys.path.insert(0, "/opt/trn_rl_repo")

import concourse.bass as bass
import concourse.mybir as mybir
import concourse.tile as tile
from concourse import bacc
from concourse.bass_utils import run_bass_kernel_spmd

F32 = mybir.dt.float32
BF16 = mybir.dt.bfloat16

B, DIM, H, W = 4, 256, 128, 128
INNER, P = 256, 8
HEADS, HG = 8, 4          # total heads, heads per group
C = INNER // HEADS        # 32 head channels
HWF = H * W               # 16384 flattened positions
KEYS = (H // P) * (W // P)  # 256 patches
N_CORES = 8
PT = 512                  # position tile
NPT = HWF // PT           # 32 position tiles

_CACHE = {}
_EYE = np.ascontiguousarray(np.tile(np.eye(64, dtype=np.float32), (2, 1)))


def _build():
    nc = bacc.Bacc(trn_type="TRN2", target_bir_lowering=False, debug=False)

    x_d = nc.dram_tensor("x", [DIM, HWF], BF16, kind="ExternalInput")
    wq_d = nc.dram_tensor("wq", [DIM, 128], BF16, kind="ExternalInput")
    # [ (ck*8 + chunk)*128 + p , s8*128 + m ]
    wk_d = nc.dram_tensor("wk", [DIM * 8, 1024], BF16, kind="ExternalInput")
    wv_d = nc.dram_tensor("wv", [DIM * 8, 1024], BF16, kind="ExternalInput")
    bq_d = nc.dram_tensor("bq", [128, 1], F32, kind="ExternalInput")
    bk_d = nc.dram_tensor("bk", [128, 1], F32, kind="ExternalInput")
    bv_d = nc.dram_tensor("bv", [128, 1], F32, kind="ExternalInput")
    eye_d = nc.dram_tensor("eye", [128, 64], F32, kind="ExternalInput")
    # pair p output: rows 0:32 num(h=2p), 32 rowsum(h=2p), 64:96 num(2p+1),
    # 96 rowsum(2p+1); other rows are garbage.
    od0_d = nc.dram_tensor("od0", [128, HWF], BF16, kind="ExternalOutput")
    od1_d = nc.dram_tensor("od1", [128, HWF], BF16, kind="ExternalOutput")


    SPLIT = 6  # tiles that run A/exp per key-half (early start) with C deferred

    with tile.TileContext(nc) as tc:
        with tc.tile_pool(name="const", bufs=1) as cw, \
             tc.tile_pool(name="xq", bufs=1) as xq, \
             tc.tile_pool(name="wkv", bufs=3) as wkv, \
             tc.tile_pool(name="qt", bufs=SPLIT + 3) as qt, \
             tc.tile_pool(name="ep", bufs=6) as ep, \
             tc.tile_pool(name="es", bufs=4 * SPLIT) as es, \
             tc.tile_pool(name="op", bufs=3) as op, \
             tc.tile_pool(name="pa", bufs=3, space="PSUM") as pa, \
             tc.tile_pool(name="po", bufs=2, space="PSUM") as po:

            # x arrives as 4 quarters [chunk ck, pos-half]: the kb=0 key
            # half of the k conv only needs the first 8192 positions, so
            # attention (A + exp on kb0) starts ~20us before x fully lands.
            HH = HWF // 2
            x_t = [[None, None], [None, None]]  # [ck][half]

            def load_x(ck, half):
                xt = xq.tile([128, HH], BF16, tag=f"x{ck}{half}")
                nc.sync.dma_start(
                    xt[:], x_d.ap()[ck * 128:(ck + 1) * 128,
                                    half * HH:(half + 1) * HH])
                x_t[ck][half] = xt

            def load_w(w_d, ck):
                wt = wkv.tile([128, 8192], BF16, tag="w")
                nc.sync.dma_start(
                    wt[:].rearrange("p (chunk m) -> p chunk m", m=1024),
                    w_d.ap()[ck * 1024:(ck + 1) * 1024, :].rearrange(
                        "(chunk p) m -> p chunk m", p=128))
                return wt

            # DMA issue order IS the critical path: k-path first.
            load_x(0, 0)
            wk_t = [load_w(wk_d, 0)]
            load_x(1, 0)
            bk_sb = cw.tile([128, 1], F32)
            nc.sync.dma_start(bk_sb[:], bk_d.ap())
            wq_sb = cw.tile([128, 256], BF16)
            nc.sync.dma_start(wq_sb[:, 0:128], wq_d.ap()[0:128, :])
            nc.sync.dma_start(wq_sb[:, 128:256], wq_d.ap()[128:256, :])
            bq_sb = cw.tile([128, 1], F32)
            nc.sync.dma_start(bq_sb[:], bq_d.ap())
            wk_t.append(load_w(wk_d, 1))
            load_x(0, 1)
            load_x(1, 1)
            wv_t = [load_w(wv_d, 0), load_w(wv_d, 1)]
            bv_sb = cw.tile([128, 1], F32)
            nc.sync.dma_start(bv_sb[:], bv_d.ap())
            ident = cw.tile([128, 64], F32)
            nc.sync.dma_start(ident[:], eye_d.ap())

            v_sb = cw.tile([128, KEYS], F32)
            k_ext = cw.tile([128, 512], BF16)
            nc.vector.memset(k_ext[:], 0.0)
            # per-(h,kb) 64-col blocks: [vT(32) | ones(1) | zeros(31)] --
            # the zero cols make the C matmul define all po2 partitions.
            vt_sb = cw.tile([128, 8 * 64], BF16)
            nc.vector.memset(vt_sb[:], 0.0)
            for u in range(8):
                nc.vector.memset(vt_sb[:, u * 64 + 32:u * 64 + 33], 1.0)
            junk = cw.tile([128, 512], BF16)
            nc.vector.memset(junk[:], 0.0)

            # PE DVFS warm-up: keep the tensor engine clocked up while the
            # x/wk DMAs land. Dummies chained on the x tiles can't be
            # hoisted earlier than their DMA, pinning them into the gaps.
            scratch = po.tile([128, 512], F32, tag="po")

            def warm(n, src=None):
                t = junk if src is None else src
                for _ in range(n):
                    nc.tensor.matmul(scratch[:], t[:, 0:128], t[:, 0:512],
                                     start=True, stop=True)

            warm(13)

            def conv_part(pk, w_tiles, half, ck, ij0, ij1):
                # keys half*128 .. half*128+128 from positions of that half
                xv = x_t[ck][half][:].rearrange(
                    "p (ph i pw j) -> p i j ph pw", ph=8, i=8, pw=16, j=8)
                wt = w_tiles[ck]
                for ij in range(ij0, ij1):
                    i, j = ij // 8, ij % 8
                    off = i * 1024 + j * 128
                    nc.tensor.matmul(
                        pk[:, half * 128:(half + 1) * 128],
                        wt[:, off:off + 128], xv[:, i, j],
                        start=(ck == 0 and ij == 0),
                        stop=(ck == 1 and ij == 63))

            def k_finish(kb):
                # bias-add fused into the per-head zero-padded k_ext blocks
                hs = slice(kb * 128, (kb + 1) * 128)
                for hp in range(2):
                    for p in range(2):
                        rows = slice(hp * 64 + p * 32, hp * 64 + p * 32 + 32)
                        cols = slice((p * 2 + kb) * 128, (p * 2 + kb) * 128 + 128)
                        nc.vector.tensor_scalar_add(
                            k_ext[rows, cols], pk_k[rows, hs], bk_sb[rows])

            def make_q(pt):
                half, o = pt // 16, (pt % 16) * PT
                s2 = slice(o, o + PT)
                pq_t = pa.tile([128, PT], F32, tag="pa")
                nc.tensor.matmul(pq_t[:], wq_sb[:, 0:128], x_t[0][half][:, s2],
                                 start=True, stop=False)
                nc.tensor.matmul(pq_t[:], wq_sb[:, 128:256], x_t[1][half][:, s2],
                                 start=False, stop=True)
                q_t = qt.tile([128, PT], BF16, tag="q")
                nc.vector.tensor_scalar_add(q_t[:], pq_t[:], bq_sb[:])
                return q_t

            def a_exp_pair_half(q_t, pair, kb, e2_t):
                # dots for key block kb of BOTH heads of a pair (they share
                # the q partition range) -> one [128,1024] exp
                qs = q_t[pair * 64:(pair + 1) * 64, :]
                pa_t = pa.tile([128, 1024], F32, tag="pa")
                for p in range(2):
                    nc.tensor.matmul(
                        pa_t[:, p * 512:(p + 1) * 512],
                        k_ext[pair * 64:(pair + 1) * 64,
                              (p * 2 + kb) * 128:(p * 2 + kb) * 128 + 128],
                        qs, start=True, stop=True)
                nc.scalar.activation(e2_t[:], pa_t[:],
                                     mybir.ActivationFunctionType.Exp)

            def a_exp_full(q_t, h):
                hp, p = h // 2, h % 2
                qs = q_t[hp * 64:(hp + 1) * 64, :]
                pa_t = pa.tile([128, 1024], F32, tag="pa")
                for kb in range(2):
                    nc.tensor.matmul(
                        pa_t[:, kb * 512:(kb + 1) * 512],
                        k_ext[hp * 64:(hp + 1) * 64,
                              (p * 2 + kb) * 128:(p * 2 + kb) * 128 + 128],
                        qs, start=True, stop=True)
                e_t = ep.tile([128, 1024], BF16, tag="e")
                nc.scalar.activation(e_t[:], pa_t[:],
                                     mybir.ActivationFunctionType.Exp)
                return e_t

            def cv_pair(po2, h, refs):
                ob = (h % 2) * 64
                for kb in range(2):
                    u = h * 2 + kb
                    e_t, off = refs[kb]
                    nc.tensor.matmul(
                        po2[ob:ob + 64, :],
                        vt_sb[:, u * 64:(u + 1) * 64],
                        e_t[:, off:off + 512],
                        start=(kb == 0), stop=(kb == 1))

            def out_pair(po2, pt, pair):
                o_t = op.tile([128, PT], BF16, tag="o")
                nc.vector.tensor_copy(o_t[:], po2[:])
                od = od0_d if pair == 0 else od1_d
                nc.sync.dma_start(
                    od.ap()[:, pt * PT:(pt + 1) * PT], o_t[:])

            # ---- k conv, key-half 0 -> early attention ----
            pk_k = po.tile([128, KEYS], F32, tag="po")
            warm(21, x_t[0][0])
            conv_part(pk_k, wk_t, 0, 0, 0, 64)
            # fill the x1a/wk1 DMA gap with dummies, then real q-convs as
            # soon as x1a lands (they need only x-a + wq)
            warm(28, x_t[0][0])
            q_pre = [make_q(i) for i in range(6)]
            conv_part(pk_k, wk_t, 0, 1, 0, 64)
            k_finish(0)

            # phase A: A/exp on kb0 for the first SPLIT tiles, with the
            # key-half-1 conv woven in so PE computes it while Act paces
            pk_v = po.tile([128, KEYS], F32, tag="po")
            kb_chunks = [(0, 0, 16), (0, 16, 32), (0, 32, 48), (0, 48, 64),
                         (1, 0, 32), (1, 32, 64)]
            stq, ste = [], []
            q_cur = None
            for pt in range(SPLIT):
                q_t = q_pre[pt] if pt < len(q_pre) else q_cur
                stq.append(q_t)
                els = [[None, None], [None, None]]   # [pair][kb]
                for pair in range(2):
                    e2 = es.tile([128, 1024], BF16, tag="es",
                                 name=f"ea{pt}_{pair}")
                    a_exp_pair_half(q_t, pair, 0, e2)
                    els[pair][0] = e2
                ste.append(els)
                if pt + 1 >= len(q_pre):
                    q_cur = make_q(pt + 1)
                if pt < len(kb_chunks):
                    ck, a, b = kb_chunks[pt]
                    conv_part(pk_k, wk_t, 1, ck, a, b)
            for ck, a, b in kb_chunks[SPLIT:]:
                conv_part(pk_k, wk_t, 1, ck, a, b)
            k_finish(1)

            # ---- phase B: kb1 backfill exps, v conv woven in ----
            # half-h with ck0 first within each half; ck1 weights land later
            # half 0 must fully close (stop flag) before half 1 opens:
            # both halves share one PSUM zero region. 16-matmul chunks,
            # 2 per phase-B tile; the tail spills into early steady tiles.
            vv_all = [(h, ck, o, o + 16)
                      for h in range(2) for ck in range(2)
                      for o in range(0, 64, 16)]
            vv_sched = [[] for _ in range(SPLIT)]
            _n = [3, 3, 3, 3, 2, 2] + [0] * SPLIT
            _i = 0
            for t in range(SPLIT):
                vv_sched[t] = vv_all[_i:_i + _n[t]]
                _i += _n[t]
            vv_sched[-1] += vv_all[_i:]
            for pt in range(SPLIT):
                for pair in range(2):
                    e2 = es.tile([128, 1024], BF16, tag="es",
                                 name=f"eb{pt}_{pair}")
                    a_exp_pair_half(stq[pt], pair, 1, e2)
                    ste[pt][pair][1] = e2
                for half, ck, a, b in vv_sched[pt]:
                    conv_part(pk_v, wv_t, half, ck, a, b)
            stq = None

            def v_finish():
                nc.vector.tensor_scalar_add(v_sb[:], pk_v[:], bv_sb[:])
                for hp in range(2):
                    for kb in range(2):
                        ptr = po.tile([128, 64], F32, tag="po",
                                      name=f"ptr{hp}{kb}")
                        nc.tensor.transpose(
                            ptr[:],
                            v_sb[hp * 64:(hp + 1) * 64, kb * 128:(kb + 1) * 128],
                            ident[hp * 64:(hp + 1) * 64, :])
                        for p in range(2):
                            h = hp * 2 + p
                            u = h * 2 + kb
                            nc.vector.tensor_copy(
                                vt_sb[:, u * 64:u * 64 + 32],
                                ptr[:, p * 32:(p + 1) * 32])

            v_finish()

            # ---- steady state; deferred C: one full pair per catch step
            # (self-contained so PSUM recycling never crosses a pending
            # accumulation) ----
            catch = [(pt, pair) for pt in range(SPLIT) for pair in range(2)]

            def do_catch():
                if not catch:
                    return
                pt0, pair0 = catch.pop(0)
                po2c = po.tile([128, PT], F32, tag="po", name=f"ca{pt0}_{pair0}")
                for p in range(2):
                    cv_pair(po2c, 2 * pair0 + p,
                            [(ste[pt0][pair0][0], p * 512),
                             (ste[pt0][pair0][1], p * 512)])
                out_pair(po2c, pt0, pair0)

            for pt in range(SPLIT, NPT):
                q_t = q_cur
                es = [a_exp_full(q_t, h) for h in range(HG)]
                if pt + 1 < NPT:
                    q_cur = make_q(pt + 1)
                for pair in range(2):
                    po2 = po.tile([128, PT], F32, tag="po")
                    for p in range(2):
                        et = es[2 * pair + p]
                        cv_pair(po2, 2 * pair + p, [(et, 0), (et, 512)])
                    out_pair(po2, pt, pair)
                if pt > SPLIT:
                    do_catch()
            while catch:
                do_catch()

    nc.compile()
    return nc


def _head_index(g):
    # device row m = h_local*32 + c_idx  ->  full-channel c_idx*8 + 4g + h_local
    m = np.arange(128)
    return (m % 32) * 8 + 4 * g + (m // 32)


def _prep_wkv(Wf, idx):
    # -> [ck, chunk, p, s8, m] flattened to [2048, 1024], bf16
    t = np.asarray(Wf, np.float32)[idx]          # [m, cin, i, j]
    t = t.reshape(128, 2, 128, 64)               # m, ck, p, ij
    t = t.transpose(1, 3, 2, 0)                  # ck, ij, p, m
    t = t.reshape(2, 8, 8, 128, 128)             # ck, chunk, s8, p, m
    t = t.transpose(0, 1, 3, 2, 4)               # ck, chunk, p, s8, m
    return np.ascontiguousarray(t.reshape(2048, 1024))


def kernel(x, Wq, bq, Wk, bk, Wv, bv):
    import ml_dtypes
    bf16 = ml_dtypes.bfloat16
    if "nc" not in _CACHE:
        _CACHE["nc"] = _build()
    nc = _CACHE["nc"]

    x = np.asarray(x, np.float32)
    in_maps = []
    idxs = []
    for g in range(2):
        idx = _head_index(g)
        idxs.append(idx)
        wq_h = np.ascontiguousarray(Wq[idx, :, 0, 0].T).astype(bf16)  # [256,128]
        wk_h = _prep_wkv(Wk, idx).astype(bf16)
        wv_h = _prep_wkv(Wv, idx).astype(bf16)
        bq_h = np.ascontiguousarray(bq[idx].reshape(128, 1), np.float32)
        bk_h = np.ascontiguousarray(bk[idx].reshape(128, 1), np.float32)
        bv_h = np.ascontiguousarray(bv[idx].reshape(128, 1), np.float32)
        for b in range(B):
            in_maps.append({
                "x": np.ascontiguousarray(x[b].reshape(DIM, HWF)).astype(bf16),
                "wq": wq_h, "wk": wk_h, "wv": wv_h,
                "bq": bq_h, "bk": bk_h, "bv": bv_h,
                "eye": _EYE,
            })
    # core order: core = b*2 + g  -> reorder in_maps built as g-major
    order = [g * B + b for b in range(B) for g in range(2)]
    in_maps = [in_maps[i] for i in order]

    res = run_bass_kernel_spmd(nc, in_maps, core_ids=list(range(N_CORES)))
    _CACHE["last"] = res

    out = np.empty((B, INNER, H, W), np.float32)
    num = np.empty((128, HWF), np.float32)
    rs = np.empty((4, HWF), np.float32)
    for core in range(N_CORES):
        b, g = core // 2, core % 2
        od = (res.results[core]["od0"], res.results[core]["od1"])
        for h in range(4):
            blk = np.asarray(od[h // 2][(h % 2) * 64:(h % 2) * 64 + 33],
                             np.float32)
            num[h * 32:(h + 1) * 32] = blk[0:32]
            rs[h] = blk[32]
        full = num.reshape(4, 32, HWF) / rs[:, None, :]
        out[b, idxs[g]] = full.reshape(128, H, W).reshape(128, H, W)
    return out
